# revision 16
# baseline (speedup 1.0000x reference)
"""Trainium2 Bass kernel for nn_MultiHeadAttention_59614146068609.

Sharding: 8 cores = 2 batches x 4 head-groups (4 heads each).
Each core projects q/k/v for its batch with its head-slice of Wq/Wk/Wv
(column-sharded), runs causal+padded attention for its 4 heads, and
applies its row-slice of Wo, producing a partial [D, S] fp16 output.
The host sums the 4 partials per batch and adds bo.

v4: single software-pipelined emission keeps the PE tensor engine
continuously busy (idle gaps also drop the PE to a low p-state, so they
cost double):
  - inputs are host-repacked so every DMA is one dma_start with 128
    partition-major descriptors (descriptor writing at ~0.3ns/desc plus
    ~1us fixed per call was the startup bottleneck); x arrives as one
    call per (tensor, 512-col chunk) into persistent SBUF chunk tiles,
    issued upfront in consumption-priority order across the sync /
    scalar / gpsimd sequencers;
  - projections for chunk s+1 are interleaved front-loaded and the Wo
    output matmuls for chunk s-1 back-loaded as PE filler inside the
    attention loop of chunk s, so Scalar-engine exp latency never stalls
    the PE and next-chunk prerequisites are ready on time; the last few
    Wo fillers are held past the final attention loop so the PE has work
    while the last softmax normalization drains on DVE;
  - scores are computed transposed (S.T[k, q], k on partitions); the
    V tile's columns DK:128 are all ones, so the PV matmul's psum rows
    64:128 hold the softmax denominator replicated 64x -- normalization
    is copy + reciprocal + multiply on DVE (the copy to SBUF is
    load-bearing: the custom-DVE reciprocal misreads PSUM on hardware);
  - QK matmuls use 64-partition contraction slices directly, so the
    upper halves of q/k tiles never need zeroing;
  - exp for a head-pair runs as one Scalar activation over a 2-bank
    psum tile (halves per-call overhead); padding mask is its bias;
  - output is staged per q-chunk into one [128, 8, 512] fp16 tile and
    written with a single dma_start per chunk; end-of-kernel staging
    casts run on the then-idle Scalar engine instead of DVE.

The kernel is specialized at build time on kb_cap = number of 128-wide
key blocks that contain any unpadded key; fully padded key blocks are
skipped entirely.
"""

import numpy as np

S = 2048
B = 2
D = 1024
H = 16
DK = 64
N_CORES = 8
GROUPS = N_CORES // B          # head groups per batch = 4
HPG = H // GROUPS              # heads per group = 4
OC = HPG * DK                  # per-core projected dim = 256
OT = OC // 128                 # o-tiles per core = 2
IT = D // 128                  # contraction tiles = 8
SC = S // 512                  # sequence chunks of 512 = 4
KB = S // 128                  # k blocks of 128 = 16
NEG = -1e30

_cache = {}


def _build_nc(kb_cap, fbm):
    import concourse.bacc as bacc
    import concourse.bass as bass
    import concourse.mybir as mybir
    import concourse.tile as tile

    F32 = mybir.dt.float32
    FP16 = mybir.dt.float16
    Exp = mybir.ActivationFunctionType.Exp
    Copy = mybir.ActivationFunctionType.Copy
    PSUM = bass.MemorySpace.PSUM

    KW = kb_cap * 128                       # valid key width
    KC = -(-KW // 512)                      # k chunks of 512
    VC = -(-kb_cap // 4)                    # v chunks

    def kcw(c):
        return min(512, KW - c * 512)

    def nkb(qc):
        return min(4 * (qc + 1), kb_cap)

    def vblocks(s):
        return range(4 * s, min(4 * s + 4, kb_cap))

    nc = bacc.Bacc("TRN2", target_bir_lowering=False, debug=False)

    # x tensors arrive host-packed as [chunk, 128, IT, 512]
    xq = nc.dram_tensor("xq", [SC, 128, IT, 512], FP16, kind="ExternalInput")
    xk = nc.dram_tensor("xk", [SC, 128, IT, 512], FP16, kind="ExternalInput")
    xv = nc.dram_tensor("xv", [SC, 128, IT, 512], FP16, kind="ExternalInput")
    # weights host-packed partition-major
    wq = nc.dram_tensor("wq", [128, IT, OC], FP16, kind="ExternalInput")
    wk = nc.dram_tensor("wk", [128, IT, OC], FP16, kind="ExternalInput")
    wv = nc.dram_tensor("wv", [128, IT, OC], FP16, kind="ExternalInput")
    wo = nc.dram_tensor("wo", [128, OT, D], FP16, kind="ExternalInput")
    bias_qk = nc.dram_tensor("bias_qk", [128, 2, OT], F32, kind="ExternalInput")
    bias_v = nc.dram_tensor("bias_v", [128, OC], F32, kind="ExternalInput")
    pad = nc.dram_tensor("pad", [128, KB], F32, kind="ExternalInput")
    causal = nc.dram_tensor("causal", [128, 2, 128], FP16, kind="ExternalInput")
    out_t = nc.dram_tensor("out_t", [D, S], FP16, kind="ExternalOutput")

    with tile.TileContext(nc) as tc, nc.allow_low_precision(
        reason="fp16 matmul inputs with fp32 accumulation; validated vs reference"
    ):
        with (
            tc.tile_pool(name="persist", bufs=1) as pp,
            tc.tile_pool(name="fill", bufs=2, space=PSUM) as ps_fill,
            tc.tile_pool(name="st", bufs=2, space=PSUM) as ps_st,
            tc.tile_pool(name="o", bufs=2, space=PSUM) as ps_o,
            tc.tile_pool(name="pt", bufs=3) as ptp,
            tc.tile_pool(name="nrm", bufs=4) as nrm,
            tc.tile_pool(name="stg", bufs=2) as stg,
        ):
            # ---- persistent SBUF tensors ----
            t_wq = pp.tile([128, IT, OC], FP16, name="t_wq")
            t_wk = pp.tile([128, IT, OC], FP16, name="t_wk")
            t_wv = pp.tile([128, IT, OC], FP16, name="t_wv")
            t_wo = pp.tile([128, OT, D], FP16, name="t_wo")
            t_bqk = pp.tile([128, 2, OT], F32, name="t_bqk")
            t_bv = pp.tile([128, OC], F32, name="t_bv")
            t_pad = pp.tile([128, KB], F32, name="t_pad")
            t_causal = pp.tile([128, 2, 128], FP16, name="t_causal")
            t_qT = [
                pp.tile([128, HPG, 512], FP16, name=f"t_qT{c}") for c in range(SC)
            ]
            t_kT = [
                pp.tile([128, HPG, 512], FP16, name=f"t_kT{c}") for c in range(KC)
            ]
            nvb = [len(vblocks(s)) for s in range(VC)]
            t_V = [
                pp.tile([128, nvb[s], HPG, 128], FP16, name=f"t_V{s}")
                for s in range(VC)
            ]
            t_OT = [
                pp.tile([128, OT, 512], FP16, name=f"t_OT{c}") for c in range(SC)
            ]
            # chunk tiles: all 8 contraction sub-tiles of one 512-col chunk.
            # chunk 0 is stored as two i-halves so the first projection
            # matmuls start after half the chunk lands.
            def xtiles(prefix, n):
                out = []
                for c in range(n):
                    if c == 0:
                        out.append(
                            [
                                pp.tile([128, IT // 2, 512], FP16, name=f"{prefix}0a"),
                                pp.tile([128, IT // 2, 512], FP16, name=f"{prefix}0b"),
                            ]
                        )
                    else:
                        out.append([pp.tile([128, IT, 512], FP16, name=f"{prefix}{c}")])
                return out

            t_xq = xtiles("t_xq", SC)
            t_xk = xtiles("t_xk", KC)
            t_xv = xtiles("t_xv", VC)

            def xslice(tiles, c, i):
                if c == 0:
                    return tiles[0][i // (IT // 2)][:, i % (IT // 2), :]
                return tiles[c][0][:, i, :]

            # ---- all input DMAs upfront, in global priority order ----
            # scalar: first-needed weights + small tensors; sync: x chunks
            # with wv/wo sequenced between them so later-stage weights do
            # not steal ring bandwidth from the first chunks.
            nc.scalar.dma_start(out=t_wk, in_=wk[:])
            nc.scalar.dma_start(out=t_wq, in_=wq[:])
            nc.scalar.dma_start(out=t_bqk, in_=bias_qk[:])
            nc.scalar.dma_start(out=t_pad, in_=pad[:])
            nc.scalar.dma_start(out=t_causal, in_=causal[:])
            nc.scalar.dma_start(out=t_bv, in_=bias_v[:])
            hi = IT // 2
            nc.sync.dma_start(out=t_xk[0][0], in_=xk[0][:, 0:hi, :])
            nc.sync.dma_start(out=t_xq[0][0], in_=xq[0][:, 0:hi, :])
            nc.sync.dma_start(out=t_xk[0][1], in_=xk[0][:, hi:IT, :])
            nc.sync.dma_start(out=t_xq[0][1], in_=xq[0][:, hi:IT, :])
            nc.sync.dma_start(out=t_xv[0][0], in_=xv[0][:, 0:hi, :])
            nc.sync.dma_start(out=t_xv[0][1], in_=xv[0][:, hi:IT, :])
            nc.sync.dma_start(out=t_wv, in_=wv[:])
            for c in range(1, SC):
                if c < KC:
                    nc.sync.dma_start(out=t_xk[c][0], in_=xk[c])
                nc.sync.dma_start(out=t_xq[c][0], in_=xq[c])
                if c < VC:
                    nc.sync.dma_start(out=t_xv[c][0], in_=xv[c])
                if c == 1:
                    nc.sync.dma_start(out=t_wo, in_=wo[:])
            # V columns DK:128 are all ones: the PV matmul then leaves the
            # softmax denominator replicated across psum rows 64:128.
            for s in range(VC):
                nc.gpsimd.memset(t_V[s][:, :, :, DK:128], 1.0)

            # ---- chain emitters (each ~0.9-1.7us of PE work) ----
            def qk_chain(which, c, ot):
                if which == "q":
                    xt, w_sb, dst, bidx, w = t_xq, t_wq, t_qT[c], 0, 512
                else:
                    xt, w_sb, dst, bidx, w = t_xk, t_wk, t_kT[c], 1, kcw(c)
                acc = ps_fill.tile(
                    [128, 512], F32, tag="fill", name=f"acc_{which}_{c}_{ot}"
                )
                for i in range(IT):
                    nc.tensor.matmul(
                        acc[:, 0:w],
                        w_sb[:, i, ot * 128 : (ot + 1) * 128],
                        xslice(xt, c, i)[:, 0:w],
                        start=(i == 0),
                        stop=(i == IT - 1),
                    )
                for half in range(2):
                    h = 2 * ot + half
                    p0 = half * 64
                    nc.vector.tensor_scalar_add(
                        out=dst[0:64, h, 0:w],
                        in0=acc[p0 : p0 + 64, 0:w],
                        scalar1=t_bqk[p0 : p0 + 64, bidx, ot : ot + 1],
                    )

            def v_chain(st_):
                s, sub = st_ // 4, st_ % 4
                vacc = ps_fill.tile([128, 512], F32, tag="fill", name=f"vacc_{st_}")
                for i in range(IT):
                    nc.tensor.matmul(
                        vacc[:, 0:OC],
                        xslice(t_xv, s, i)[:, sub * 128 : (sub + 1) * 128],
                        t_wv[:, i, :],
                        start=(i == 0),
                        stop=(i == IT - 1),
                    )
                nc.vector.tensor_add(
                    out=t_V[st_ // 4][:, st_ % 4, :, 0:DK],
                    in0=vacc[:, 0:OC].rearrange("p (h d) -> p h d", h=HPG),
                    in1=t_bv.rearrange("p (h d) -> p h d", h=HPG),
                )

            so_tiles = {}

            def c_chain(qc, dt_, cast_on_scalar=False):
                q0 = qc * 512
                if dt_ == 0:
                    so_tiles[qc] = stg.tile(
                        [128, IT, 512], FP16, tag="sg", name=f"so_{qc}"
                    )
                so = so_tiles[qc]
                ops = ps_fill.tile([128, 512], F32, tag="fill", name=f"c_{qc}_{dt_}")
                for j in range(OT):
                    nc.tensor.matmul(
                        ops,
                        t_wo[:, j, dt_ * 128 : (dt_ + 1) * 128],
                        t_OT[qc][:, j, :],
                        start=(j == 0),
                        stop=(j == OT - 1),
                    )
                if cast_on_scalar:
                    nc.scalar.activation(so[:, dt_, :], ops, func=Copy)
                else:
                    nc.vector.tensor_copy(so[:, dt_, :], ops)
                if dt_ == IT // 2 - 1:
                    nc.sync.dma_start(
                        out=out_t[0 : D // 2, q0 : q0 + 512].rearrange(
                            "(i p) s -> p i s", p=128
                        ),
                        in_=so[:, 0 : IT // 2, :],
                    )
                elif dt_ == IT - 1:
                    nc.sync.dma_start(
                        out=out_t[D // 2 : D, q0 : q0 + 512].rearrange(
                            "(i p) s -> p i s", p=128
                        ),
                        in_=so[:, IT // 2 : IT, :],
                    )

            def stage_chains(s, with_v=True):
                out = []
                if s < KC:
                    for ot in range(OT):
                        out.append(lambda c=s, o=ot: qk_chain("k", c, o))
                for ot in range(OT):
                    out.append(lambda c=s, o=ot: qk_chain("q", c, o))
                if with_v:
                    for st_ in vblocks(s):
                        out.append(lambda b=st_: v_chain(b))
                return out

            # ---- stage 0 projections, then pipelined attention ----
            # (stage-0 v chains are emitted inside qc0's first pass)
            for ch in stage_chains(0, with_v=False):
                ch()

            for qc in range(SC):
                q0 = qc * 512
                nb = nkb(qc)
                last_qc = qc == SC - 1
                iters = 2 * nb
                # schedule fillers per attention iteration: next-stage
                # projections front-loaded, previous chunk's Wo back-loaded
                sched = [[] for _ in range(iters)]
                pre = stage_chains(qc + 1) if qc + 1 < SC else []
                post = (
                    [
                        (lambda q_=qc - 1, d_=d_: c_chain(q_, d_))
                        for d_ in range(D // 128)
                    ]
                    if qc >= 1
                    else []
                )
                if pre:
                    win = max(1, min(iters, (iters * 2) // 5))
                    for j, ch in enumerate(pre):
                        sched[min(win - 1, (j * win) // len(pre))].append(ch)
                if post:
                    w0 = min(max(1, min(iters, (iters * 2) // 5)), 2 * len(pre)) if pre else 0
                    span = max(1, iters - w0)
                    for j, ch in enumerate(post):
                        sched[min(iters - 1, w0 + (j * span) // len(post))].append(ch)
                it_i = 0
                for pair in ((0, 1), (2, 3)):
                    o_ps = {
                        h: ps_o.tile([128, 512], F32, tag="o", name=f"o_{qc}_{h}")
                        for h in pair
                    }

                    def emit_pv(kb, off, pt2):
                        for n, h in enumerate(pair):
                            nc.tensor.matmul(
                                o_ps[h][:, off:512],
                                t_V[kb // 4][:, kb % 4, h, :],
                                pt2[:, n, off:512],
                                start=(kb == 0),
                                stop=(kb == nb - 1),
                            )

                    lagged = None
                    for kb in range(nb):
                        k0 = kb * 128
                        off = max(0, k0 - q0)
                        st2 = ps_st.tile(
                            [128, 2, 512], F32, tag="st", name=f"st_{qc}_{pair[0]}_{kb}"
                        )
                        for n, h in enumerate(pair):
                            nc.tensor.matmul(
                                st2[:, n, off:512],
                                t_kT[kb // 4][0:64, h, (kb % 4) * 128 : (kb % 4) * 128 + 128],
                                t_qT[qc][0:64, h, off:512],
                                start=True,
                                stop=True,
                            )
                        if k0 >= q0:
                            nc.vector.tensor_add(
                                out=st2[:, :, off : off + 128],
                                in0=st2[:, :, off : off + 128],
                                in1=t_causal,
                            )
                        pt2 = ptp.tile(
                            [128, 2, 512], FP16, tag="pt", name=f"pt_{qc}_{pair[0]}_{kb}"
                        )
                        nc.scalar.activation(
                            out=pt2[:, :, off:512],
                            in_=st2[:, :, off:512],
                            func=Exp,
                            bias=t_pad[:, kb : kb + 1],
                            scale=1.0,
                        )
                        if qc == 0 and pair == (0, 1):
                            # V projection for this block lands just in time;
                            # emitting it here (before its PV) starts the
                            # attention pipeline ~5us earlier
                            v_chain(kb)
                        # PV for the previous block: its pt has been ready for
                        # a full iteration, so the PE never waits on exp here
                        if lagged is not None:
                            emit_pv(*lagged)
                        lagged = (kb, off, pt2)
                        for ch in sched[it_i]:
                            ch()
                        it_i += 1
                    if lagged is not None:
                        emit_pv(*lagged)
                    final_pair = last_qc and pair == (2, 3)
                    for h in pair:
                        ot, p0 = h // 2, (h % 2) * 64
                        # copy the replicated denominator rows to SBUF first:
                        # the custom-DVE reciprocal misreads PSUM on hardware.
                        t_l = nrm.tile([64, 512], F32, tag="l", name=f"l_{qc}_{h}")
                        if final_pair:
                            # scalar is idle after the last exp; shorten the
                            # serial DVE tail
                            nc.scalar.activation(t_l, o_ps[h][64:128, :], func=Copy)
                        else:
                            nc.vector.tensor_copy(t_l, o_ps[h][64:128, :])
                        t_r = nrm.tile([64, 512], F32, tag="r", name=f"r_{qc}_{h}")
                        nc.vector.reciprocal_approx_fast(t_r, t_l)
                        nc.vector.tensor_mul(
                            t_OT[qc][p0 : p0 + 64, ot, :],
                            o_ps[h][0:64, :],
                            t_r,
                        )

            for d_ in range(D // 128):
                c_chain(SC - 1, d_, cast_on_scalar=(d_ % 2 == 0))
    nc.compile()
    return nc


def _get_nc(kb_cap, fbm):
    key = ("nc", kb_cap, fbm)
    if key not in _cache:
        _cache[key] = _build_nc(kb_cap, fbm)
    return _cache[key]


def _pack_chunks(xt):
    """[D, S] -> [SC, 128, IT, 512] contiguous (partition-major chunks)."""
    return np.ascontiguousarray(
        xt.reshape(IT, 128, SC, 512).transpose(2, 1, 0, 3)
    )


def kernel(
    query,
    key,
    value,
    Wq,
    bq,
    Wk,
    bk,
    Wv,
    bv,
    Wo,
    bo,
    attn_mask,
    key_padding_mask,
):
    from concourse import bass_utils

    query = np.asarray(query, dtype=np.float32)
    key = np.asarray(key, dtype=np.float32)
    value = np.asarray(value, dtype=np.float32)
    Wq = np.asarray(Wq, dtype=np.float32)
    bq = np.asarray(bq, dtype=np.float32)
    Wk = np.asarray(Wk, dtype=np.float32)
    bk = np.asarray(bk, dtype=np.float32)
    Wv = np.asarray(Wv, dtype=np.float32)
    bv = np.asarray(bv, dtype=np.float32)
    Wo = np.asarray(Wo, dtype=np.float32)
    bo = np.asarray(bo, dtype=np.float32)
    attn_mask = np.asarray(attn_mask)
    key_padding_mask = np.asarray(key_padding_mask)

    # this kernel hardcodes the causal structure of attn_mask
    expected = np.triu(np.ones((S, S), dtype=bool), k=1)
    assert np.array_equal(attn_mask, expected), "kernel assumes causal attn_mask"

    # number of 128-blocks that contain any valid (unpadded) key
    valid = ~key_padding_mask  # [B, S]
    kb_cap = 0
    fbm = KB
    for b in range(B):
        nz = np.nonzero(valid[b])[0]
        cap = (int(nz.max()) // 128 + 1) if nz.size else 1
        kb_cap = max(kb_cap, cap)
        full = valid[b].reshape(KB, 128).all(axis=1)
        lead = 0
        while lead < KB and full[lead]:
            lead += 1
        fbm = min(fbm, lead)

    scale = np.float32(1.0 / np.sqrt(DK))
    causal_tile = np.where(
        np.arange(128)[None, :] >= np.arange(128)[:, None], 0.0, -60000.0
    ).astype(np.float16)
    causal2 = np.ascontiguousarray(
        np.repeat(causal_tile[:, None, :], 2, axis=1)
    )  # [128, 2, 128]

    # per-batch packed activations (shared by the batch's 4 cores)
    xq_b = [_pack_chunks(query[:, b, :].T.astype(np.float16)) for b in range(B)]
    xk_b = [_pack_chunks(key[:, b, :].T.astype(np.float16)) for b in range(B)]
    xv_b = [_pack_chunks(value[:, b, :].T.astype(np.float16)) for b in range(B)]
    pad_b = [
        np.ascontiguousarray(
            np.where(key_padding_mask[b], NEG, 0.0)
            .astype(np.float32)
            .reshape(KB, 128)
            .T
        )
        for b in range(B)
    ]

    def pack_w(w):  # [D, OC] -> [128, IT, OC]
        return np.ascontiguousarray(w.reshape(IT, 128, OC).transpose(1, 0, 2))

    in_maps = []
    for c in range(N_CORES):
        b = c // GROUPS
        g = c % GROUPS
        o0 = g * OC
        osl = slice(o0, o0 + OC)
        bias_qk = np.stack(
            [
                (bq[osl] * scale).reshape(OT, 128).T,
                bk[osl].reshape(OT, 128).T,
            ],
            axis=1,
        ).astype(np.float32)  # [128, 2, OT]
        in_maps.append(
            {
                "xq": xq_b[b],
                "xk": xk_b[b],
                "xv": xv_b[b],
                "wq": pack_w((Wq[osl, :] * scale).T.astype(np.float16)),
                "wk": pack_w(Wk[osl, :].T.astype(np.float16)),
                "wv": pack_w(Wv[osl, :].T.astype(np.float16)),
                "wo": np.ascontiguousarray(
                    Wo[:, osl].T.astype(np.float16).reshape(OT, 128, D).transpose(1, 0, 2)
                ),
                "bias_qk": np.ascontiguousarray(bias_qk),
                "bias_v": np.ascontiguousarray(
                    np.broadcast_to(bv[osl][None, :], (128, OC)).astype(np.float32)
                ),
                "pad": pad_b[b],
                "causal": causal2,
            }
        )

    res = bass_utils.run_bass_kernel_spmd(
        _get_nc(kb_cap, fbm), in_maps, core_ids=list(range(N_CORES))
    )
    _cache["last_res"] = res

    out = np.zeros((S, B, D), dtype=np.float32)
    for b in range(B):
        acc = np.zeros((D, S), dtype=np.float32)
        for g in range(GROUPS):
            acc += res.results[b * GROUPS + g]["out_t"].astype(np.float32)
        out[:, b, :] = acc.T + bo[None, :]
    return out
